# revision 48
# baseline (speedup 1.0000x reference)
"""Trainium2 Bass kernel for nn_AutoEncoder_53781580481200 (moe_routing).

Host/device split:
  host: atoms are globally stable-sorted by symbol (the MoE routing) and
        dealt to the 8 cores in equal per-(core,symbol) slices, so every
        core runs an identical program with minimal padding (NG_s =
        ceil(ceil(C_s/8)/128)*128 per symbol, chosen at runtime from the
        data - ~2.3% less work than image-aligned sharding); x is stored
        transposed [D, NS] in bf16 (contiguous DMA rows, half the HBM
        traffic of f32). Per-(core,symbol,image) run-boundary tables
        stay host-side.
  device (per core): per-symbol 2-layer MLP + energy head, all matmuls
        bf16 at full PE rate. ReLU+bias evacuations are the true
        bottleneck (only ACT and DVE can read PSUM on TRN2; GPSIMD
        cannot, and matmul can't write 16-bit PSUM before TRN3), so the
        two stages are balanced across them: E1 (h1 = relu(W1.T x + b1))
        as per-tile ACT ops, E2 (h2) as one [128,1024] DVE op per pair
        of tiles. Energies accumulate as PSUM columns e[m,c] =
        e(atom c*128+m) via 128-column L3 matmuls (lhsT=h2 chunk,
        rhs=w3*slope) into a dedicated PSUM bank (no bank is ever
        shared between a PE write and a concurrent ACT/DVE read, which
        costs serializing semaphores on TRN2).
  host: gp = cumsum(e); per-image energies = prefix diffs at run
        boundaries + per-symbol affine constants x run counts (O(B)).

The pipeline is software-pipelined over units (pairs of tiles): engines
execute their streams in order, so the emission order skews stages
(L1(U) | E1(U-1), L2(U-1) | E2(U-2), L3(U-2)) to keep PE from blocking
on evacuations. Constants are fused into one bf16 blob -> single DMA;
the ACT activation-table load is pre-triggered by a dummy ReLU.

e_all is drained in two halves: the first mid-body once its columns are
final; the second is DEFERRED - each body drains its predecessor's tail
at body start (DVE idle, deps satisfied), and the loop is followed by
one post-loop drain for the final iteration, keeping the body tail off
the cross-iteration critical path.

build_nc(nrep=K, staggered=True) wraps the pipeline in a hardware loop
(tc.For_i with staggered reset, i.e. no full inter-iteration barrier)
so K back-to-back executions can be timed in one dispatch - this is how
test.py measures HW exec time under the ~51ms axon RPC dispatch floor.
"""

import numpy as np
import ml_dtypes

import concourse.bass as bass
import concourse.bacc as bacc
import concourse.mybir as mybir
import concourse.tile as tile
from concourse.bass_utils import run_bass_kernel_spmd

# problem constants
N, D, H, S, B = 262144, 128, 128, 4, 1024
NCORES = 8

T = 512              # atoms per full compute tile
CHUNK = 2048         # atoms per load chunk (512 KB)

# constant blob layout (bf16, [128, CB])
_W1_OFF = 0
_W2_OFF = 512
_W3_OFF = 1024
_B1_OFF = 1028
_B2_OFF = 1032
CB = 1036

F32 = mybir.dt.float32
I32 = mybir.dt.int32
BF16 = mybir.dt.bfloat16
AF = mybir.ActivationFunctionType
ALU = mybir.AluOpType


def plan(ngs):
    """Unit/e-column schedule shared by build_nc and the host. Units are
    pairs of 512-tiles (plus per-symbol remainder tiles); e-columns fill
    a dedicated PSUM bank in atom order."""
    ngs = tuple(int(g) for g in ngs)
    symbase = [0]
    for g in ngs:
        assert g % 128 == 0
        symbase.append(symbase[-1] + g)
    NS = symbase[-1]
    KC = NS // 128
    assert KC <= 512
    units = []
    col = 0
    for s in range(S):
        base = symbase[s]
        off = 0
        while ngs[s] - off >= 1024:
            t0 = (s, base + off, T, col)
            t1 = (s, base + off + T, T, col + 4)
            units.append((t0, t1))
            col += 8
            off += 1024
        rem = ngs[s] - off
        if rem > T:
            units.append(
                ((s, base + off, T, col), (s, base + off + T, rem - T, col + 4))
            )
            col += rem // 128
        elif rem:
            units.append(((s, base + off, rem, col),))
            col += rem // 128
    assert col == KC
    # first unit index by which all e-columns < KC//2 are emitted
    HC = KC // 2
    half_u = 0
    c = 0
    for u, unit in enumerate(units):
        c += sum(t[2] for t in unit) // 128
        if c >= HC:
            half_u = u
            break
    return dict(
        ngs=ngs, symbase=symbase, NS=NS, KC=KC, units=units, half_u=half_u,
    )


_LAST_NGS = None  # set by prepare_inputs; build_nc default


def build_nc(ngs=None, nrep=1, unroll=1, staggered=False):
    if ngs is None:
        ngs = _LAST_NGS
    assert ngs is not None, "call prepare_inputs first or pass ngs"
    pl = plan(ngs)
    NS, KC = pl["NS"], pl["KC"]
    units, symbase = pl["units"], pl["symbase"]
    NU = len(units)
    HC = KC // 2
    HALF_U = pl["half_u"]

    nc = bacc.Bacc()

    xst_d = nc.declare_dram_parameter("xst", [D, NS], BF16, isOutput=False)
    cst_d = nc.declare_dram_parameter("cst", [128, CB], BF16, isOutput=False)
    e_d = nc.declare_dram_parameter("e", [128, KC], F32, isOutput=True)

    with tile.TileContext(nc) as tc:
        with (
            tc.tile_pool(name="const", bufs=1) as cpool,
            tc.tile_pool(name="xload", bufs=4) as gpool,
            tc.tile_pool(name="h1", bufs=4) as h1pool,
            tc.tile_pool(name="h2", bufs=4) as h2pool,
            tc.tile_pool(name="seg", bufs=1) as spool,
            tc.tile_pool(name="ph1", bufs=3, space="PSUM") as ph1,
            tc.tile_pool(name="ph2", bufs=2, space="PSUM") as ph2,
            tc.tile_pool(name="pea", bufs=1, space="PSUM") as pea,
        ):
            # ---- ACT table preload: dummy ReLU on a zeroed tile ----
            zt = cpool.tile([128, 1], F32, tag="zt")
            nc.vector.memset(zt[:], 0.0)
            zt2 = cpool.tile([128, 1], F32, tag="zt2")
            nc.scalar.activation(out=zt2[:], in_=zt[:], func=AF.Relu)

            # ---- preload constants: one bf16 DMA ----
            cst_sb = cpool.tile([128, CB], BF16, tag="cst")
            nc.sync.dma_start(out=cst_sb[:], in_=cst_d[:])
            w1_sb = [
                cst_sb[:, _W1_OFF + 128 * s : _W1_OFF + 128 * (s + 1)]
                for s in range(S)
            ]
            w2_sb = [
                cst_sb[:, _W2_OFF + 128 * s : _W2_OFF + 128 * (s + 1)]
                for s in range(S)
            ]
            w3_sb = [cst_sb[:, _W3_OFF + s : _W3_OFF + s + 1] for s in range(S)]
            b1f = cpool.tile([128, S], F32, tag="b1f")
            nc.vector.tensor_copy(
                out=b1f[:], in_=cst_sb[:, _B1_OFF : _B1_OFF + S]
            )
            b2f = cpool.tile([128, S], F32, tag="b2f")
            nc.vector.tensor_copy(
                out=b2f[:], in_=cst_sb[:, _B2_OFF : _B2_OFF + S]
            )
            b1_sb = [b1f[:, s : s + 1] for s in range(S)]
            b2_sb = [b2f[:, s : s + 1] for s in range(S)]

            # e_all: persistent dedicated PSUM bank + SBUF staging
            e_all = pea.tile([128, KC], F32, tag="eall")
            nc.vector.memset(e_all[:], 0.0)
            e_sb = spool.tile([128, KC], F32, tag="e_sb")

            def evac(eng, out, in_, bias):
                if eng == "act":
                    nc.scalar.activation(
                        out=out, in_=in_, func=AF.Relu, bias=bias
                    )
                else:
                    nc.vector.tensor_scalar(
                        out=out, in0=in_, scalar1=bias, scalar2=0.0,
                        op0=ALU.add, op1=ALU.max,
                    )

            def drain_tail():
                # second e half: deferred to the next body / post-loop
                nc.vector.tensor_copy(out=e_sb[:, HC:], in_=e_all[:, HC:])
                nc.sync.dma_start(out=e_d[:, HC:], in_=e_sb[:, HC:])

            def body():
                h1_ps_u, h2_ps_u, h2_sb_u = {}, {}, {}
                xch = {}

                def load_chunk(s, ci):
                    if (s, ci) in xch:
                        return
                    base = symbase[s] + ci * CHUNK
                    sz = min(CHUNK, ngs[s] - ci * CHUNK)
                    xt = gpool.tile([128, CHUNK], BF16, tag="xtc")
                    nc.sync.dma_start(
                        out=xt[:, :sz], in_=xst_d[:, base : base + sz]
                    )
                    xch[(s, ci)] = xt

                # carry: drain the predecessor body's e tail while this
                # body's pipeline fills
                drain_tail()

                for U in range(NU + 3):
                    # L1 for unit U
                    if U < NU:
                        tiles = []
                        for (s, off, sz, _c) in units[U]:
                            woff = off - symbase[s]
                            ci, co = divmod(woff, CHUNK)
                            load_chunk(s, ci)
                            h1_ps = ph1.tile([128, T], F32, tag="h1_ps")
                            nc.tensor.matmul(
                                out=h1_ps[:, :sz], lhsT=w1_sb[s],
                                rhs=xch[(s, ci)][:, co : co + sz],
                                start=True, stop=True,
                            )
                            tiles.append(h1_ps)
                        h1_ps_u[U] = tiles
                    # E1 + L2 for unit U-1 (E1 as ACT singles so L2 of the
                    # first tile starts while the second evacuates)
                    Um = U - 1
                    if 0 <= Um < NU:
                        unit = units[Um]
                        usz = sum(t[2] for t in unit)
                        h1_sb = h1pool.tile([128, 2 * T], BF16, tag="h1_sb")
                        h2_ps = ph2.tile([128, 2 * T], F32, tag="h2_ps")
                        lo = 0
                        for (s, off, sz, _c), h1_ps in zip(unit, h1_ps_u.pop(Um)):
                            evac(
                                "act", h1_sb[:, lo : lo + sz],
                                h1_ps[:, :sz], b1_sb[s],
                            )
                            nc.tensor.matmul(
                                out=h2_ps[:, lo : lo + sz], lhsT=w2_sb[s],
                                rhs=h1_sb[:, lo : lo + sz],
                                start=True, stop=True,
                            )
                            lo += sz
                        h2_ps_u[Um] = h2_ps
                    # E2 (one DVE op per unit) for unit U-2
                    Um = U - 2
                    if 0 <= Um < NU:
                        unit = units[Um]
                        usz = sum(t[2] for t in unit)
                        s0 = unit[0][0]
                        h2_sb = h2pool.tile([128, 2 * T], BF16, tag="h2_sb")
                        h2_ps = h2_ps_u.pop(Um)
                        evac("dve", h2_sb[:, :usz], h2_ps[:, :usz], b2_sb[s0])
                        h2_sb_u[Um] = h2_sb
                    # L3 for unit U-3 (one step behind E2 so the in-order
                    # PE stream doesn't block on a same-step DVE op)
                    Um = U - 3
                    if 0 <= Um < NU:
                        unit = units[Um]
                        h2_sb = h2_sb_u.pop(Um)
                        lo = 0
                        for (s, off, sz, c0) in unit:
                            for j in range(sz // 128):
                                nc.tensor.matmul(
                                    out=e_all[:, c0 + j : c0 + j + 1],
                                    lhsT=h2_sb[:, lo + j * 128 : lo + (j + 1) * 128],
                                    rhs=w3_sb[s],
                                    start=True, stop=True,
                                )
                            lo += sz
                    # first-half e evacuation as soon as its columns final
                    if U == HALF_U + 4:
                        nc.vector.tensor_copy(
                            out=e_sb[:, :HC], in_=e_all[:, :HC]
                        )
                    if U == HALF_U + 6:
                        nc.sync.dma_start(out=e_d[:, :HC], in_=e_sb[:, :HC])

            if nrep == 1:
                body()
            elif nrep == unroll:
                for _ in range(nrep):
                    body()
            else:
                assert nrep % unroll == 0
                with tc.For_i(0, nrep // unroll, 1, staggered_reset=staggered):
                    for _ in range(unroll):
                        body()
            # final iteration's deferred tail drain
            drain_tail()
    nc.finalize()
    return nc


def prepare_inputs(x, symbol_ids, image_ids, W1, b1, W2, b2, W3, b3, slope,
                   intercept):
    """Global stable sort by symbol, equal per-(core,symbol) dealing;
    run-boundary tables kept host-side. Returns (in_maps, metas)."""
    global _LAST_NGS
    x = np.ascontiguousarray(np.asarray(x, dtype=np.float32))
    sym = np.asarray(symbol_ids, dtype=np.int32)
    img = np.asarray(image_ids, dtype=np.int32)
    W1 = np.ascontiguousarray(np.asarray(W1, np.float32))
    W2 = np.ascontiguousarray(np.asarray(W2, np.float32))
    W3 = np.asarray(W3, np.float32)
    b1 = np.ascontiguousarray(np.asarray(b1, np.float32))
    b2 = np.ascontiguousarray(np.asarray(b2, np.float32))
    b3 = np.asarray(b3, np.float32)
    slope = np.asarray(slope, np.float32)
    intercept = np.asarray(intercept, np.float32)

    W3c = (W3 * slope[:, None]).astype(np.float32)
    cvec = (slope * b3 + intercept).astype(np.float32).reshape(1, S)

    cst = np.zeros((128, CB), ml_dtypes.bfloat16)
    for s in range(S):
        cst[:, _W1_OFF + 128 * s : _W1_OFF + 128 * (s + 1)] = W1[s]
        cst[:, _W2_OFF + 128 * s : _W2_OFF + 128 * (s + 1)] = W2[s]
        cst[:, _W3_OFF + s] = W3c[s]
        cst[:, _B1_OFF + s] = b1[s]
        cst[:, _B2_OFF + s] = b2[s]

    order = np.argsort(sym, kind="stable").astype(np.int64)
    counts = np.bincount(sym, minlength=S)
    starts = np.concatenate([[0], np.cumsum(counts)])
    gs = [(int(counts[s]) + NCORES - 1) // NCORES for s in range(S)]
    ngs = tuple((g + 127) // 128 * 128 for g in gs)
    _LAST_NGS = ngs
    pl = plan(ngs)
    NS, symbase = pl["NS"], pl["symbase"]

    in_maps, metas = [], []
    for k in range(NCORES):
        xs = np.zeros((NS, D), ml_dtypes.bfloat16)
        bnd = np.zeros(S * (B + 1), np.int64)
        cnts = np.zeros((S, B), np.int64)
        for s in range(S):
            lo = starts[s] + k * gs[s]
            hi = min(starts[s] + (k + 1) * gs[s], starts[s + 1])
            gidx = order[lo:hi]
            cnt = hi - lo
            base = symbase[s]
            xs[base : base + cnt] = x[gidx]
            gimg = img[gidx]
            ends = np.searchsorted(gimg, np.arange(B), "right")
            bnd[s * (B + 1) : s * (B + 1) + B] = base + ends - 1
            bnd[s * (B + 1) + B] = base + ngs[s] - 1
            cnts[s] = np.diff(np.concatenate([[0], ends]))
        xst = np.ascontiguousarray(xs.T)  # [D, NS] bf16
        in_maps.append(dict(xst=xst, cst=cst))
        metas.append((bnd, cnts, cvec))
    return in_maps, (metas, pl)


def finish_output(results, metas):
    """Per-image energies from device per-atom energies: host prefix sums +
    O(B) boundary diffs."""
    metas, pl = metas
    KC = pl["KC"]
    out = np.zeros(B, np.float32)
    for k in range(NCORES):
        bnd, cnts, cvec = metas[k]
        e2d = np.asarray(results[k]["e"], np.float64)  # [128, KC]
        gp = np.cumsum(e2d.T.reshape(-1))
        q = bnd
        gpv = np.where(q >= 0, gp[np.maximum(q, 0)], 0.0)
        t = np.concatenate([[0.0], gpv])
        rs = (t[1:] - t[:-1]).reshape(S, B + 1)[:, :B]
        rs = rs + cvec.reshape(S, 1) * cnts  # per-symbol affine constants
        out += rs.sum(axis=0).astype(np.float32)
    return out


_NC_CACHE = {}


def kernel(**inputs):
    in_maps, metas = prepare_inputs(**inputs)
    ngs = metas[1]["ngs"]
    if ngs not in _NC_CACHE:
        _NC_CACHE[ngs] = build_nc(ngs)
    res = run_bass_kernel_spmd(_NC_CACHE[ngs], in_maps, list(range(NCORES)))
    return finish_output(res.results, metas)


# revision 50
# speedup vs baseline: 1.0675x; 1.0675x over previous
"""Trainium2 Bass kernel for nn_AutoEncoder_53781580481200 (moe_routing).

Host/device split:
  host: atoms are globally stable-sorted by symbol (the MoE routing) and
        dealt to the 8 cores in equal per-(core,symbol) slices, so every
        core runs an identical program with minimal padding (NG_s =
        ceil(ceil(C_s/8)/128)*128 per symbol, chosen at runtime from the
        data - ~2.3% less work than image-aligned sharding); x is stored
        transposed [D, NS] in bf16 (contiguous DMA rows, half the HBM
        traffic of f32). Per-(core,symbol,image) run-boundary tables
        stay host-side.
  device (per core): per-symbol 2-layer MLP + energy head, all matmuls
        bf16 at full PE rate. ReLU+bias evacuations are the true
        bottleneck (only ACT and DVE can read PSUM on TRN2; GPSIMD
        cannot, and matmul can't write 16-bit PSUM before TRN3), so the
        two stages are balanced across them: E1 (h1 = relu(W1.T x + b1))
        as per-tile ACT ops, E2 (h2) as one [128,1024] DVE op per pair
        of tiles. Energies accumulate as PSUM columns e[m,c] =
        e(atom c*128+m) via 128-column L3 matmuls (lhsT=h2 chunk,
        rhs=w3*slope) into a dedicated PSUM bank (no bank is ever
        shared between a PE write and a concurrent ACT/DVE read, which
        costs serializing semaphores on TRN2).
  host: gp = cumsum(e); per-image energies = prefix diffs at run
        boundaries + per-symbol affine constants x run counts (O(B)).

The pipeline is software-pipelined over units (pairs of tiles): engines
execute their streams in order, so the emission order skews stages
(L1(U) | E1(U-1), L2(U-1) | E2(U-2), L3(U-2)) to keep PE from blocking
on evacuations. Constants are fused into one bf16 blob -> single DMA;
the ACT activation-table load is pre-triggered by a dummy ReLU.

e_all is drained in two halves: the first mid-body once its columns are
final; the second is DEFERRED - each body drains its predecessor's tail
at body start (DVE idle, deps satisfied), and the loop is followed by
one post-loop drain for the final iteration, keeping the body tail off
the cross-iteration critical path.

build_nc(nrep=K, staggered=True) wraps the pipeline in a hardware loop
(tc.For_i with staggered reset, i.e. no full inter-iteration barrier)
so K back-to-back executions can be timed in one dispatch - this is how
test.py measures HW exec time under the ~51ms axon RPC dispatch floor.
"""

import numpy as np
import ml_dtypes

import concourse.bass as bass
import concourse.bacc as bacc
import concourse.mybir as mybir
import concourse.tile as tile
from concourse.bass_utils import run_bass_kernel_spmd

# problem constants
N, D, H, S, B = 262144, 128, 128, 4, 1024
NCORES = 8

T = 512              # atoms per full compute tile
CHUNK = 2048         # atoms per load chunk (512 KB)

# constant blob layout (bf16, [128, CB])
_W1_OFF = 0
_W2_OFF = 512
_W3_OFF = 1024
_B1_OFF = 1028
_B2_OFF = 1032
CB = 1036

F32 = mybir.dt.float32
I32 = mybir.dt.int32
BF16 = mybir.dt.bfloat16
AF = mybir.ActivationFunctionType
ALU = mybir.AluOpType


def plan(ngs):
    """Unit/e-column schedule shared by build_nc and the host. Units are
    pairs of 512-tiles (plus per-symbol remainder tiles); e-columns fill
    a dedicated PSUM bank in atom order."""
    ngs = tuple(int(g) for g in ngs)
    symbase = [0]
    for g in ngs:
        assert g % 128 == 0
        symbase.append(symbase[-1] + g)
    NS = symbase[-1]
    KC = NS // 128
    assert KC <= 512
    units = []
    col = 0
    for s in range(S):
        base = symbase[s]
        off = 0
        while ngs[s] - off >= 1024:
            t0 = (s, base + off, T, col)
            t1 = (s, base + off + T, T, col + 4)
            units.append((t0, t1))
            col += 8
            off += 1024
        rem = ngs[s] - off
        if rem > T:
            units.append(
                ((s, base + off, T, col), (s, base + off + T, rem - T, col + 4))
            )
            col += rem // 128
        elif rem:
            units.append(((s, base + off, rem, col),))
            col += rem // 128
    assert col == KC
    # first unit index by which all e-columns < KC//2 are emitted
    HC = KC // 2
    half_u = 0
    c = 0
    for u, unit in enumerate(units):
        c += sum(t[2] for t in unit) // 128
        if c >= HC:
            half_u = u
            break
    return dict(
        ngs=ngs, symbase=symbase, NS=NS, KC=KC, units=units, half_u=half_u,
    )


_LAST_NGS = None  # set by prepare_inputs; build_nc default


def build_nc(ngs=None, nrep=1, unroll=1, staggered=False):
    if ngs is None:
        ngs = _LAST_NGS
    assert ngs is not None, "call prepare_inputs first or pass ngs"
    pl = plan(ngs)
    NS, KC = pl["NS"], pl["KC"]
    units, symbase = pl["units"], pl["symbase"]
    NU = len(units)
    HC = KC // 2
    HALF_U = pl["half_u"]

    nc = bacc.Bacc()

    xst_d = nc.declare_dram_parameter("xst", [D, NS], BF16, isOutput=False)
    cst_d = nc.declare_dram_parameter("cst", [128, CB], BF16, isOutput=False)
    e_d = nc.declare_dram_parameter("e", [128, KC], F32, isOutput=True)

    with tile.TileContext(nc) as tc:
        with (
            tc.tile_pool(name="const", bufs=1) as cpool,
            tc.tile_pool(name="xload", bufs=4) as gpool,
            tc.tile_pool(name="h1", bufs=4) as h1pool,
            tc.tile_pool(name="h2", bufs=4) as h2pool,
            tc.tile_pool(name="seg", bufs=1) as spool,
            tc.tile_pool(name="ph1", bufs=3, space="PSUM") as ph1,
            tc.tile_pool(name="ph2", bufs=2, space="PSUM") as ph2,
            tc.tile_pool(name="pea", bufs=1, space="PSUM") as pea,
        ):
            # ---- ACT table preload: dummy ReLU on a zeroed tile ----
            zt = cpool.tile([128, 1], F32, tag="zt")
            nc.vector.memset(zt[:], 0.0)
            zt2 = cpool.tile([128, 1], F32, tag="zt2")
            nc.scalar.activation(out=zt2[:], in_=zt[:], func=AF.Relu)

            # ---- preload constants: one bf16 DMA ----
            cst_sb = cpool.tile([128, CB], BF16, tag="cst")
            nc.sync.dma_start(out=cst_sb[:], in_=cst_d[:])
            w1_sb = [
                cst_sb[:, _W1_OFF + 128 * s : _W1_OFF + 128 * (s + 1)]
                for s in range(S)
            ]
            w2_sb = [
                cst_sb[:, _W2_OFF + 128 * s : _W2_OFF + 128 * (s + 1)]
                for s in range(S)
            ]
            w3_sb = [cst_sb[:, _W3_OFF + s : _W3_OFF + s + 1] for s in range(S)]
            b1f = cpool.tile([128, S], F32, tag="b1f")
            nc.vector.tensor_copy(
                out=b1f[:], in_=cst_sb[:, _B1_OFF : _B1_OFF + S]
            )
            b2f = cpool.tile([128, S], F32, tag="b2f")
            nc.vector.tensor_copy(
                out=b2f[:], in_=cst_sb[:, _B2_OFF : _B2_OFF + S]
            )
            b1_sb = [b1f[:, s : s + 1] for s in range(S)]
            b2_sb = [b2f[:, s : s + 1] for s in range(S)]

            # e_all: persistent dedicated PSUM bank + SBUF staging
            e_all = pea.tile([128, KC], F32, tag="eall")
            nc.vector.memset(e_all[:], 0.0)
            e_sb = spool.tile([128, KC], F32, tag="e_sb")

            def evac(eng, out, in_, bias):
                if eng == "act":
                    nc.scalar.activation(
                        out=out, in_=in_, func=AF.Relu, bias=bias
                    )
                else:
                    nc.vector.tensor_scalar(
                        out=out, in0=in_, scalar1=bias, scalar2=0.0,
                        op0=ALU.add, op1=ALU.max,
                    )

            def drain_tail():
                # second e half: deferred to the next body / post-loop
                nc.vector.tensor_copy(out=e_sb[:, HC:], in_=e_all[:, HC:])
                nc.sync.dma_start(out=e_d[:, HC:], in_=e_sb[:, HC:])

            def body():
                h1_ps_u, h2_ps_u, h2_sb_u = {}, {}, {}
                xch = {}

                def load_chunk(s, ci):
                    if (s, ci) in xch:
                        return
                    base = symbase[s] + ci * CHUNK
                    sz = min(CHUNK, ngs[s] - ci * CHUNK)
                    xt = gpool.tile([128, CHUNK], BF16, tag="xtc")
                    nc.sync.dma_start(
                        out=xt[:, :sz], in_=xst_d[:, base : base + sz]
                    )
                    xch[(s, ci)] = xt

                # carry: drain the predecessor body's e tail while this
                # body's pipeline fills
                drain_tail()

                for U in range(NU + 3):
                    # L1 for unit U
                    if U < NU:
                        tiles = []
                        for (s, off, sz, _c) in units[U]:
                            woff = off - symbase[s]
                            ci, co = divmod(woff, CHUNK)
                            load_chunk(s, ci)
                            h1_ps = ph1.tile([128, T], F32, tag="h1_ps")
                            nc.tensor.matmul(
                                out=h1_ps[:, :sz], lhsT=w1_sb[s],
                                rhs=xch[(s, ci)][:, co : co + sz],
                                start=True, stop=True,
                            )
                            tiles.append(h1_ps)
                        h1_ps_u[U] = tiles
                    # E1 + L2 for unit U-1 (E1 as ACT singles so L2 of the
                    # first tile starts while the second evacuates)
                    Um = U - 1
                    if 0 <= Um < NU:
                        unit = units[Um]
                        usz = sum(t[2] for t in unit)
                        h1_sb = h1pool.tile([128, 2 * T], BF16, tag="h1_sb")
                        h2_ps = ph2.tile([128, 2 * T], F32, tag="h2_ps")
                        lo = 0
                        for (s, off, sz, _c), h1_ps in zip(unit, h1_ps_u.pop(Um)):
                            evac(
                                "act", h1_sb[:, lo : lo + sz],
                                h1_ps[:, :sz], b1_sb[s],
                            )
                            nc.tensor.matmul(
                                out=h2_ps[:, lo : lo + sz], lhsT=w2_sb[s],
                                rhs=h1_sb[:, lo : lo + sz],
                                start=True, stop=True,
                            )
                            lo += sz
                        h2_ps_u[Um] = h2_ps
                    # E2 (one DVE op per unit) for unit U-2
                    Um = U - 2
                    if 0 <= Um < NU:
                        unit = units[Um]
                        usz = sum(t[2] for t in unit)
                        s0 = unit[0][0]
                        h2_sb = h2pool.tile([128, 2 * T], BF16, tag="h2_sb")
                        h2_ps = h2_ps_u.pop(Um)
                        evac("dve", h2_sb[:, :usz], h2_ps[:, :usz], b2_sb[s0])
                        h2_sb_u[Um] = h2_sb
                    # L3 for unit U-3 (one step behind E2 so the in-order
                    # PE stream doesn't block on a same-step DVE op)
                    Um = U - 3
                    if 0 <= Um < NU:
                        unit = units[Um]
                        h2_sb = h2_sb_u.pop(Um)
                        lo = 0
                        for (s, off, sz, c0) in unit:
                            for j in range(sz // 128):
                                nc.tensor.matmul(
                                    out=e_all[:, c0 + j : c0 + j + 1],
                                    lhsT=h2_sb[:, lo + j * 128 : lo + (j + 1) * 128],
                                    rhs=w3_sb[s],
                                    start=True, stop=True,
                                )
                            lo += sz
                    # first-half e evacuation as soon as its columns final
                    if U == HALF_U + 4:
                        nc.vector.tensor_copy(
                            out=e_sb[:, :HC], in_=e_all[:, :HC]
                        )
                    if U == HALF_U + 6:
                        nc.sync.dma_start(out=e_d[:, :HC], in_=e_sb[:, :HC])

            if nrep == 1:
                body()
            elif nrep == unroll:
                for _ in range(nrep):
                    body()
            else:
                assert nrep % unroll == 0
                with tc.For_i(0, nrep // unroll, 1, staggered_reset=staggered):
                    for _ in range(unroll):
                        body()
            # final iteration's deferred tail drain
            drain_tail()
    nc.finalize()
    return nc


def prepare_inputs(x, symbol_ids, image_ids, W1, b1, W2, b2, W3, b3, slope,
                   intercept):
    """Global stable sort by symbol, equal per-(core,symbol) dealing;
    run-boundary tables kept host-side. Returns (in_maps, metas)."""
    global _LAST_NGS
    x = np.ascontiguousarray(np.asarray(x, dtype=np.float32))
    sym = np.asarray(symbol_ids, dtype=np.int32)
    img = np.asarray(image_ids, dtype=np.int32)
    W1 = np.ascontiguousarray(np.asarray(W1, np.float32))
    W2 = np.ascontiguousarray(np.asarray(W2, np.float32))
    W3 = np.asarray(W3, np.float32)
    b1 = np.ascontiguousarray(np.asarray(b1, np.float32))
    b2 = np.ascontiguousarray(np.asarray(b2, np.float32))
    b3 = np.asarray(b3, np.float32)
    slope = np.asarray(slope, np.float32)
    intercept = np.asarray(intercept, np.float32)

    W3c = (W3 * slope[:, None]).astype(np.float32)
    cvec = (slope * b3 + intercept).astype(np.float32).reshape(1, S)

    cst = np.zeros((128, CB), ml_dtypes.bfloat16)
    for s in range(S):
        cst[:, _W1_OFF + 128 * s : _W1_OFF + 128 * (s + 1)] = W1[s]
        cst[:, _W2_OFF + 128 * s : _W2_OFF + 128 * (s + 1)] = W2[s]
        cst[:, _W3_OFF + s] = W3c[s]
        cst[:, _B1_OFF + s] = b1[s]
        cst[:, _B2_OFF + s] = b2[s]

    order = np.argsort(sym, kind="stable").astype(np.int64)
    counts = np.bincount(sym, minlength=S)
    starts = np.concatenate([[0], np.cumsum(counts)])
    gs = [(int(counts[s]) + NCORES - 1) // NCORES for s in range(S)]
    ngs = tuple((g + 127) // 128 * 128 for g in gs)
    _LAST_NGS = ngs
    pl = plan(ngs)
    NS, symbase = pl["NS"], pl["symbase"]

    in_maps, metas = [], []
    for k in range(NCORES):
        xs = np.zeros((NS, D), ml_dtypes.bfloat16)
        bnd = np.zeros(S * (B + 1), np.int64)
        cnts = np.zeros((S, B), np.int64)
        for s in range(S):
            lo = starts[s] + k * gs[s]
            hi = min(starts[s] + (k + 1) * gs[s], starts[s + 1])
            gidx = order[lo:hi]
            cnt = hi - lo
            base = symbase[s]
            xs[base : base + cnt] = x[gidx]
            gimg = img[gidx]
            ends = np.searchsorted(gimg, np.arange(B), "right")
            bnd[s * (B + 1) : s * (B + 1) + B] = base + ends - 1
            bnd[s * (B + 1) + B] = base + ngs[s] - 1
            cnts[s] = np.diff(np.concatenate([[0], ends]))
        xst = np.ascontiguousarray(xs.T)  # [D, NS] bf16
        in_maps.append(dict(xst=xst, cst=cst))
        metas.append((bnd, cnts, cvec))
    return in_maps, (metas, pl)


def finish_output(results, metas):
    """Per-image energies from device per-atom energies: host prefix sums +
    O(B) boundary diffs."""
    metas, pl = metas
    KC = pl["KC"]
    out = np.zeros(B, np.float32)
    for k in range(NCORES):
        bnd, cnts, cvec = metas[k]
        e2d = np.asarray(results[k]["e"], np.float64)  # [128, KC]
        gp = np.cumsum(e2d.T.reshape(-1))
        q = bnd
        gpv = np.where(q >= 0, gp[np.maximum(q, 0)], 0.0)
        t = np.concatenate([[0.0], gpv])
        rs = (t[1:] - t[:-1]).reshape(S, B + 1)[:, :B]
        rs = rs + cvec.reshape(S, 1) * cnts  # per-symbol affine constants
        out += rs.sum(axis=0).astype(np.float32)
    return out


_NC_CACHE = {}


def kernel(**inputs):
    in_maps, metas = prepare_inputs(**inputs)
    ngs = metas[1]["ngs"]
    if ngs not in _NC_CACHE:
        _NC_CACHE[ngs] = build_nc(ngs)
    res = run_bass_kernel_spmd(_NC_CACHE[ngs], in_maps, list(range(NCORES)))
    return finish_output(res.results, metas)
